# revision 6
# baseline (speedup 1.0000x reference)
"""Trainium2 Bass kernel for nn_Attention (additive/Bahdanau-style attention).

Math (reference):
    enc [S,B,2H] -> [B,S,2H]
    energy  = tanh(h @ Wh^T + enc @ We^T + b)    # [B,S,H]
    logits  = energy . v                         # [B,S]
    out     = softmax(logits, axis=S)            # [B,S]

Sharding: data-parallel over batch. B=16 rows over 8 NeuronCores -> 2 rows
per core; attn weights replicated. No collectives needed.

Per-core device layout ("T" = feature-major so the softmax row sits on one
partition and the tanh bias is per-partition):
    enc  [2, 2048, 1024]  = enc[s, b, e] pre-transposed on host to [b, e, s]
    wet  [2048, 1024]     = We^T (lhsT for the main matmul)
    wht  [1024, 1024]     = Wh^T
    ht   [1024, 2]        = hidden rows, transposed
    bt   [128, 8]         = attn_b tiled per 128-partition chunk
    vt   [128, 8]         = v tiled per 128-partition chunk
Main matmul: e_projT[o, s] accumulated over K=2048 in PSUM (fp32r PE path),
ScalarE fuses bias-add + tanh, v-dot contracts the partition dim back on the
PE, softmax runs on a [2, 1024] tile.
"""

from contextlib import ExitStack

import numpy as np

import concourse.bacc as bacc
import concourse.mybir as mybir
import concourse.tile as tile
from concourse.bass_utils import run_bass_kernel_spmd

H = 1024
B = 16
S = 1024
E = 2 * H
NCORES = 8
BL = B // NCORES        # 2 batch rows per core

PT = 128                # partition tile
NT = 512                # free-dim tile (one fp32 PSUM bank)
KT_E = E // PT          # 16 K-tiles in the main matmul
MT = H // PT            # 8 output-feature tiles
ST = S // NT            # 2 seq chunks
KT_H = H // PT          # 8 K-tiles for h_proj

F32 = mybir.dt.float32
AF = mybir.ActivationFunctionType

# "f32r": full-rate fp32 PE datapath; "f32": exact 1/4-rate fp32
COMPUTE_DTYPE = "f32r"
# "pe": v-dot as PE matmul in COMPUTE_DTYPE; "dve": exact fp32 DVE scale+add
# with a single fp32 ones-matmul partition reduce
VDOT_MODE = "pe"


def build(compute_dtype=COMPUTE_DTYPE, vdot_mode=VDOT_MODE):
    cdt = {"f32r": mybir.dt.float32r, "f32": F32}[compute_dtype]
    nc = bacc.Bacc("TRN2", target_bir_lowering=False, debug=False)

    enc = nc.dram_tensor("enc", [BL, E, S], cdt, kind="ExternalInput").ap()
    wet = nc.dram_tensor("wet", [E, H], cdt, kind="ExternalInput").ap()
    wht = nc.dram_tensor("wht", [H, H], F32, kind="ExternalInput").ap()
    ht = nc.dram_tensor("ht", [H, BL], F32, kind="ExternalInput").ap()
    bt = nc.dram_tensor("bt", [PT, MT], F32, kind="ExternalInput").ap()
    vt = nc.dram_tensor("vt", [PT, MT], cdt, kind="ExternalInput").ap()
    out = nc.dram_tensor("out", [BL, S], F32, kind="ExternalOutput").ap()

    with tile.TileContext(nc) as tc, ExitStack() as ctx:
        constp = ctx.enter_context(tc.tile_pool(name="constp", bufs=1))
        wetp = ctx.enter_context(tc.tile_pool(name="wetp", bufs=KT_E))
        whtp = ctx.enter_context(tc.tile_pool(name="whtp", bufs=KT_H))
        encp = ctx.enter_context(tc.tile_pool(name="encp", bufs=2 * KT_E))
        hpbp = ctx.enter_context(tc.tile_pool(name="hpbp", bufs=MT))
        engp = ctx.enter_context(tc.tile_pool(name="engp", bufs=3))
        accp = ctx.enter_context(tc.tile_pool(name="accp", bufs=2))
        attp = ctx.enter_context(tc.tile_pool(name="attp", bufs=1))
        smp = ctx.enter_context(tc.tile_pool(name="smp", bufs=1))
        psum_h = ctx.enter_context(tc.tile_pool(name="psum_h", bufs=2, space="PSUM"))
        psum_e = ctx.enter_context(tc.tile_pool(name="psum_e", bufs=2, space="PSUM"))
        psum_a = ctx.enter_context(tc.tile_pool(name="psum_a", bufs=2, space="PSUM"))

        # ---- constants -------------------------------------------------
        bt_sb = constp.tile([PT, MT], F32)
        nc.sync.dma_start(bt_sb[:], bt[:])
        vt_sb = constp.tile([PT, MT], cdt)
        nc.sync.dma_start(vt_sb[:], vt[:])
        ht_sb = constp.tile([PT, KT_H * BL], F32)
        nc.sync.dma_start(
            ht_sb[:].rearrange("p (k j) -> p k j", j=BL),
            ht.rearrange("(k p) j -> p k j", p=PT),
        )
        if vdot_mode == "dve":
            ones_sb = constp.tile([PT, 1], F32)
            nc.gpsimd.memset(ones_sb[:], 1.0)

        # ---- resident weights -----------------------------------------
        wet_tiles = []
        for kt in range(KT_E):
            t = wetp.tile([PT, H], cdt)
            nc.sync.dma_start(t[:], wet[kt * PT : (kt + 1) * PT, :])
            wet_tiles.append(t)
        wht_tiles = []
        for kt in range(KT_H):
            t = whtp.tile([PT, H], F32)
            nc.sync.dma_start(t[:], wht[kt * PT : (kt + 1) * PT, :])
            wht_tiles.append(t)

        # ---- phase A: hpb[o, b] = Wh @ h + attn_b (per-partition bias) --
        hpb = []
        for mt in range(MT):
            ph = psum_h.tile([PT, BL], F32)
            for kt in range(KT_H):
                nc.tensor.matmul(
                    ph[:],
                    wht_tiles[kt][:, mt * PT : (mt + 1) * PT],
                    ht_sb[:, kt * BL : (kt + 1) * BL],
                    start=(kt == 0),
                    stop=(kt == KT_H - 1),
                )
            hb = hpbp.tile([PT, BL], F32)
            nc.scalar.activation(hb[:], ph[:], AF.Identity, bias=bt_sb[:, mt : mt + 1])
            hpb.append(hb)

        # ---- phase B: main matmul + tanh + v-dot, per (b, s-chunk) ------
        # att lives on partition 0 only: compute-engine APs must start at a
        # quarter-partition boundary, so batch rows go side-by-side in the
        # free dim instead of on partitions 0/1.
        att_sb = attp.tile([1, BL * S], F32)
        for b in range(BL):
            for st in range(ST):
                etiles = []
                for kt in range(KT_E):
                    t = encp.tile([PT, NT], cdt)
                    nc.sync.dma_start(
                        t[:],
                        enc[b, kt * PT : (kt + 1) * PT, st * NT : (st + 1) * NT],
                    )
                    etiles.append(t)

                pa = psum_a.tile([1, NT], F32)
                acc = accp.tile([PT, NT], F32) if vdot_mode == "dve" else None
                for mt in range(MT):
                    pe = psum_e.tile([PT, NT], F32)
                    for kt in range(KT_E):
                        nc.tensor.matmul(
                            pe[:],
                            wet_tiles[kt][:, mt * PT : (mt + 1) * PT],
                            etiles[kt][:],
                            start=(kt == 0),
                            stop=(kt == KT_E - 1),
                        )
                    en = engp.tile([PT, NT], cdt)
                    nc.scalar.activation(
                        en[:], pe[:], AF.Tanh, bias=hpb[mt][:, b : b + 1]
                    )
                    if vdot_mode == "pe":
                        nc.tensor.matmul(
                            pa[:],
                            vt_sb[:, mt : mt + 1],
                            en[:],
                            start=(mt == 0),
                            stop=(mt == MT - 1),
                        )
                    else:
                        if mt == 0:
                            nc.vector.tensor_scalar_mul(
                                acc[:], en[:].bitcast(F32), vt_sb[:, 0:1].bitcast(F32)
                            )
                        else:
                            tmp = engp.tile([PT, NT], F32, tag="vtmp")
                            nc.vector.tensor_scalar_mul(
                                tmp[:], en[:].bitcast(F32),
                                vt_sb[:, mt : mt + 1].bitcast(F32)
                            )
                            nc.vector.tensor_add(acc[:], acc[:], tmp[:])
                if vdot_mode == "dve":
                    nc.tensor.matmul(
                        pa[:], ones_sb[:, 0:1], acc[:], start=True, stop=True
                    )
                nc.scalar.copy(
                    att_sb[0:1, b * S + st * NT : b * S + (st + 1) * NT], pa[:]
                )

        # ---- phase C: softmax over s, one [1, S] row per batch ----------
        res = smp.tile([1, BL * S], F32, tag="res")
        for b in range(BL):
            row = att_sb[0:1, b * S : (b + 1) * S]
            nmx = smp.tile([1, 1], F32, tag="nmx")
            nc.vector.reduce_max(
                nmx[:], row, axis=mybir.AxisListType.X, negate=True
            )
            ex = smp.tile([1, S], F32, tag="ex")
            nc.scalar.activation(ex[:], row, AF.Exp, bias=nmx[:])
            sm = smp.tile([1, 1], F32, tag="sm")
            nc.vector.reduce_sum(sm[:], ex[:], axis=mybir.AxisListType.X)
            rs = smp.tile([1, 1], F32, tag="rs")
            nc.vector.reciprocal(rs[:], sm[:])
            nc.vector.tensor_scalar_mul(
                res[0:1, b * S : (b + 1) * S], ex[:], rs[:]
            )
        for b in range(BL):
            nc.sync.dma_start(out[b : b + 1, :], res[0:1, b * S : (b + 1) * S])

    nc.compile()
    return nc


_NC_CACHE = {}


def _get_nc(compute_dtype=COMPUTE_DTYPE, vdot_mode=VDOT_MODE):
    key = (compute_dtype, vdot_mode)
    if key not in _NC_CACHE:
        _NC_CACHE[key] = build(compute_dtype, vdot_mode)
    return _NC_CACHE[key]


def make_in_maps(hidden_state, encoder_outputs, attn_w, attn_b, v):
    hidden_state = np.asarray(hidden_state, dtype=np.float32)
    encoder_outputs = np.asarray(encoder_outputs, dtype=np.float32)
    attn_w = np.asarray(attn_w, dtype=np.float32)
    attn_b = np.asarray(attn_b, dtype=np.float32)
    v = np.asarray(v, dtype=np.float32)

    wet_t = np.ascontiguousarray(attn_w[:, H:].T)            # [2048, 1024]
    wht_t = np.ascontiguousarray(attn_w[:, :H].T)            # [1024, 1024]
    enc_t = np.ascontiguousarray(encoder_outputs.transpose(1, 2, 0))  # [16,2048,1024]
    bt_t = np.ascontiguousarray(attn_b.reshape(MT, PT).T)    # [128, 8]
    vt_t = np.ascontiguousarray(v.reshape(MT, PT).T)         # [128, 8]

    in_maps = []
    for i in range(NCORES):
        rows = slice(i * BL, (i + 1) * BL)
        in_maps.append(
            {
                "enc": enc_t[rows],
                "wet": wet_t,
                "wht": wht_t,
                "ht": np.ascontiguousarray(hidden_state[rows].T),
                "bt": bt_t,
                "vt": vt_t,
            }
        )
    return in_maps


def run(inputs, trace=False, compute_dtype=COMPUTE_DTYPE, vdot_mode=VDOT_MODE,
        **spmd_kwargs):
    nc = _get_nc(compute_dtype, vdot_mode)
    in_maps = make_in_maps(**inputs)
    res = run_bass_kernel_spmd(
        nc, in_maps, core_ids=list(range(NCORES)), trace=trace, **spmd_kwargs
    )
    out = np.concatenate([res.results[i]["out"] for i in range(NCORES)], axis=0)
    return out.astype(np.float32), res


def kernel(**inputs):
    out, _ = run(inputs, trace=False)
    return out


# revision 9
# speedup vs baseline: 1.1585x; 1.1585x over previous
"""Trainium2 Bass kernel for nn_Attention (additive/Bahdanau-style attention).

Math (reference):
    enc [S,B,2H] -> [B,S,2H]
    energy  = tanh(h @ Wh^T + enc @ We^T + b)    # [B,S,H]
    logits  = energy . v                         # [B,S]
    out     = softmax(logits, axis=S)            # [B,S]

Sharding: data-parallel over batch. B=16 rows over 8 NeuronCores -> 2 rows
per core; attn weights replicated. No collectives needed.

Per-core device layout ("T" = feature-major so the softmax row sits on one
partition and the tanh bias is per-partition):
    enc  [2, 2048, 1024]  = enc[s, b, e] pre-transposed on host to [b, e, s]
    wet  [2048, 1024]     = We^T (lhsT for the main matmul)
    wht  [1024, 1024]     = Wh^T
    ht   [1024, 2]        = hidden rows, transposed
    bt   [128, 8]         = attn_b tiled per 128-partition chunk
    vt   [128, 8]         = v tiled per 128-partition chunk
Main matmul: e_projT[o, s] accumulated over K=2048 in PSUM (fp32r PE path),
ScalarE fuses bias-add + tanh, v-dot contracts the partition dim back on the
PE, softmax runs on a [2, 1024] tile.
"""

from contextlib import ExitStack

import numpy as np

import concourse.bacc as bacc
import concourse.mybir as mybir
import concourse.tile as tile
from concourse.bass_utils import run_bass_kernel_spmd

H = 1024
B = 16
S = 1024
E = 2 * H
NCORES = 8
BL = B // NCORES        # 2 batch rows per core

PT = 128                # partition tile
NT = 512                # free-dim tile (one fp32 PSUM bank)
KT_E = E // PT          # 16 K-tiles in the main matmul
MT = H // PT            # 8 output-feature tiles
ST = S // NT            # 2 seq chunks
KT_H = H // PT          # 8 K-tiles for h_proj

F32 = mybir.dt.float32
AF = mybir.ActivationFunctionType

# "f32r": full-rate fp32 PE datapath; "f32": exact 1/4-rate fp32
COMPUTE_DTYPE = "f32r"
# "pe": v-dot as PE matmul in COMPUTE_DTYPE; "dve": exact fp32 DVE scale+add
# with a single fp32 ones-matmul partition reduce
VDOT_MODE = "pe"


def build(compute_dtype=COMPUTE_DTYPE, vdot_mode=VDOT_MODE):
    cdt = {"f32r": mybir.dt.float32r, "f32": F32}[compute_dtype]
    nc = bacc.Bacc("TRN2", target_bir_lowering=False, debug=False)

    enc = nc.dram_tensor("enc", [BL, E, S], cdt, kind="ExternalInput").ap()
    wet = nc.dram_tensor("wet", [E, H], cdt, kind="ExternalInput").ap()
    wht = nc.dram_tensor("wht", [H, H], cdt, kind="ExternalInput").ap()
    ht = nc.dram_tensor("ht", [H, BL], cdt, kind="ExternalInput").ap()
    bt = nc.dram_tensor("bt", [PT, MT], F32, kind="ExternalInput").ap()
    vt = nc.dram_tensor("vt", [PT, MT], cdt, kind="ExternalInput").ap()
    out = nc.dram_tensor("out", [BL, S], F32, kind="ExternalOutput").ap()

    with tile.TileContext(nc) as tc, ExitStack() as ctx:
        constp = ctx.enter_context(tc.tile_pool(name="constp", bufs=1))
        wetp = ctx.enter_context(tc.tile_pool(name="wetp", bufs=KT_E))
        whtp = ctx.enter_context(tc.tile_pool(name="whtp", bufs=KT_H))
        encp = ctx.enter_context(tc.tile_pool(name="encp", bufs=2 * KT_E))
        hpbp = ctx.enter_context(tc.tile_pool(name="hpbp", bufs=MT))
        engp = ctx.enter_context(tc.tile_pool(name="engp", bufs=3))
        attp = ctx.enter_context(tc.tile_pool(name="attp", bufs=1))
        smp = ctx.enter_context(tc.tile_pool(name="smp", bufs=1))
        # one shared PSUM pool: every tile takes one bank-sized slot, so
        # block 0 can hold all 8 accumulation groups at once
        psp = ctx.enter_context(tc.tile_pool(name="psp", bufs=8, space="PSUM"))

        # ---- constants -------------------------------------------------
        bt_sb = constp.tile([PT, MT], F32)
        nc.sync.dma_start(bt_sb[:], bt[:])
        vt_sb = constp.tile([PT, MT], cdt)
        nc.sync.dma_start(vt_sb[:], vt[:])
        ht_sb = constp.tile([PT, KT_H * BL], cdt)
        nc.sync.dma_start(
            ht_sb[:].rearrange("p (k j) -> p k j", j=BL),
            ht.rearrange("(k p) j -> p k j", p=PT),
        )

        # ---- resident weights -----------------------------------------
        # wht/ht first (phase A warms the PE while wet+enc stream in);
        # wet DMAs are interleaved with enc block 0 below.
        wht_tiles = []
        for kt in range(KT_H):
            t = whtp.tile([PT, H], cdt)
            nc.sync.dma_start(t[:], wht[kt * PT : (kt + 1) * PT, :])
            wht_tiles.append(t)

        # ---- phase A: hpb[o, b] = Wh @ h + attn_b (per-partition bias) --
        hpb = []
        for mt in range(MT):
            ph = psp.tile([PT, BL], F32, tag="ps")
            for kt in range(KT_H):
                nc.tensor.matmul(
                    ph[:],
                    wht_tiles[kt][:, mt * PT : (mt + 1) * PT],
                    ht_sb[:, kt * BL : (kt + 1) * BL],
                    start=(kt == 0),
                    stop=(kt == KT_H - 1),
                )
            hb = hpbp.tile([PT, BL], F32)
            nc.scalar.activation(hb[:], ph[:], AF.Identity, bias=bt_sb[:, mt : mt + 1])
            hpb.append(hb)

        # ---- phase B: main matmul + tanh + v-dot ------------------------
        # att lives on partition 0 only: compute-engine APs must start at a
        # quarter-partition boundary, so batch rows go side-by-side in the
        # free dim instead of on partitions 0/1.
        att_sb = attp.tile([1, BL * S], F32)

        def load_enc_tiles(b, st):
            ts = []
            for kt in range(KT_E):
                t = encp.tile([PT, NT], cdt)
                nc.sync.dma_start(
                    t[:],
                    enc[b, kt * PT : (kt + 1) * PT, st * NT : (st + 1) * NT],
                )
                ts.append(t)
            return ts

        def tanh_vdot(pes_mt, pa, b, mt):
            en = engp.tile([PT, NT], cdt)
            nc.scalar.activation(
                en[:], pes_mt[:], AF.Tanh, bias=hpb[mt][:, b : b + 1]
            )
            nc.tensor.matmul(
                pa[:],
                vt_sb[:, mt : mt + 1],
                en[:],
                start=(mt == 0),
                stop=(mt == MT - 1),
            )

        def att_store(pa, b, st):
            nc.scalar.copy(
                att_sb[0:1, b * S + st * NT : b * S + (st + 1) * NT], pa[:]
            )

        # block (0, 0): kt-outer with one DMA "pair" (wet[kt], enc[kt]) per
        # step so the PE consumes each pair right as it lands.
        wet_tiles = [None] * KT_E
        etiles = [None] * KT_E
        pes = [
            psp.tile([PT, NT], F32, tag="ps", name=f"pes{mt}") for mt in range(MT)
        ]
        for kt in range(KT_E):
            wt = wetp.tile([PT, H], cdt)
            nc.sync.dma_start(wt[:], wet[kt * PT : (kt + 1) * PT, :])
            wet_tiles[kt] = wt
            t = encp.tile([PT, NT], cdt)
            nc.sync.dma_start(t[:], enc[0, kt * PT : (kt + 1) * PT, 0:NT])
            etiles[kt] = t
            for mt in range(MT):
                nc.tensor.matmul(
                    pes[mt][:],
                    wet_tiles[kt][:, mt * PT : (mt + 1) * PT],
                    etiles[kt][:],
                    start=(kt == 0),
                    stop=(kt == KT_E - 1),
                )
        pa = psp.tile([1, NT], F32, tag="ps")
        for mt in range(MT):
            tanh_vdot(pes[mt], pa, 0, mt)
        att_store(pa, 0, 0)

        # remaining blocks: mt-outer, tanh of group mt overlaps group mt+1
        for b, st in [(0, 1), (1, 0), (1, 1)]:
            etiles = load_enc_tiles(b, st)
            pa = psp.tile([1, NT], F32, tag="ps")
            for mt in range(MT):
                pe = psp.tile([PT, NT], F32, tag="ps")
                for kt in range(KT_E):
                    nc.tensor.matmul(
                        pe[:],
                        wet_tiles[kt][:, mt * PT : (mt + 1) * PT],
                        etiles[kt][:],
                        start=(kt == 0),
                        stop=(kt == KT_E - 1),
                    )
                tanh_vdot(pe, pa, b, mt)
            att_store(pa, b, st)

        # ---- phase C: softmax over s, one [1, S] row per batch ----------
        res = smp.tile([1, BL * S], F32, tag="res")
        for b in range(BL):
            row = att_sb[0:1, b * S : (b + 1) * S]
            nmx = smp.tile([1, 1], F32, tag="nmx")
            nc.vector.reduce_max(
                nmx[:], row, axis=mybir.AxisListType.X, negate=True
            )
            ex = smp.tile([1, S], F32, tag="ex")
            nc.scalar.activation(ex[:], row, AF.Exp, bias=nmx[:])
            sm = smp.tile([1, 1], F32, tag="sm")
            nc.vector.reduce_sum(sm[:], ex[:], axis=mybir.AxisListType.X)
            rs = smp.tile([1, 1], F32, tag="rs")
            nc.vector.reciprocal(rs[:], sm[:])
            nc.vector.tensor_scalar_mul(
                res[0:1, b * S : (b + 1) * S], ex[:], rs[:]
            )
        for b in range(BL):
            nc.sync.dma_start(out[b : b + 1, :], res[0:1, b * S : (b + 1) * S])

    nc.compile()
    return nc


_NC_CACHE = {}


def _get_nc(compute_dtype=COMPUTE_DTYPE, vdot_mode=VDOT_MODE):
    key = (compute_dtype, vdot_mode)
    if key not in _NC_CACHE:
        _NC_CACHE[key] = build(compute_dtype, vdot_mode)
    return _NC_CACHE[key]


def make_in_maps(hidden_state, encoder_outputs, attn_w, attn_b, v):
    hidden_state = np.asarray(hidden_state, dtype=np.float32)
    encoder_outputs = np.asarray(encoder_outputs, dtype=np.float32)
    attn_w = np.asarray(attn_w, dtype=np.float32)
    attn_b = np.asarray(attn_b, dtype=np.float32)
    v = np.asarray(v, dtype=np.float32)

    wet_t = np.ascontiguousarray(attn_w[:, H:].T)            # [2048, 1024]
    wht_t = np.ascontiguousarray(attn_w[:, :H].T)            # [1024, 1024]
    enc_t = np.ascontiguousarray(encoder_outputs.transpose(1, 2, 0))  # [16,2048,1024]
    bt_t = np.ascontiguousarray(attn_b.reshape(MT, PT).T)    # [128, 8]
    vt_t = np.ascontiguousarray(v.reshape(MT, PT).T)         # [128, 8]

    in_maps = []
    for i in range(NCORES):
        rows = slice(i * BL, (i + 1) * BL)
        in_maps.append(
            {
                "enc": enc_t[rows],
                "wet": wet_t,
                "wht": wht_t,
                "ht": np.ascontiguousarray(hidden_state[rows].T),
                "bt": bt_t,
                "vt": vt_t,
            }
        )
    return in_maps


def run(inputs, trace=False, compute_dtype=COMPUTE_DTYPE, vdot_mode=VDOT_MODE,
        **spmd_kwargs):
    nc = _get_nc(compute_dtype, vdot_mode)
    in_maps = make_in_maps(**inputs)
    res = run_bass_kernel_spmd(
        nc, in_maps, core_ids=list(range(NCORES)), trace=trace, **spmd_kwargs
    )
    out = np.concatenate([res.results[i]["out"] for i in range(NCORES)], axis=0)
    return out.astype(np.float32), res


def kernel(**inputs):
    out, _ = run(inputs, trace=False)
    return out
